# revision 7
# baseline (speedup 1.0000x reference)
"""Diagonal Mahalanobis distance kernel for Trainium2 (8 NeuronCores, SPMD).

d2[n, m] = sum_d (s_d * (x[n,d] - y[m,d]))^2
         = ||xs_n||^2 + ||ys_m||^2 - 2 * xs @ ys^T,   xs = x*s, ys = y*s, s = exp(log_scale)

Sharding: 4x2 grid — x rows split 4 ways, y rows (output cols) split 2 ways.
Core c = (a, b): x rows [a*2048, (a+1)*2048), y rows [b*4096, (b+1)*4096).
Each core computes a (2048, 4096) tile of the distance matrix.

The kernel is HBM-store-bound (32MB f32 output per core ~ 94us at the 358GB/s
per-NC limit), so everything else is sized to stay under that:
  - Inputs are quantized to fp8 e4m3 on the host with the exp(log_scale) scale
    folded in (xs, ys -> qx, qy). Input DMA is ~3MB/core.
  - Row norms ||qx||^2 / ||qy||^2 are computed on host in f32 *from the
    quantized values*, so the device computes exactly ||qx - qy||^2: the only
    error vs the reference is the input quantization itself (~3.4e-3 fro).
  - The GEMM runs in fp8 with perf_mode=DoubleRow (2 fp8 weights per PE cell):
    operands live in SBUF as [128, k_subtile, free] and each matmul contracts
    a K=256 pair of subtiles (~36us PE busy for the 2048x4096x512 tile).
  - yn lands replicated across partitions via a K=1 ones*(-0.5) matmul
    (psum broadcast), giving ynh = -yn/2 in SBUF.
  - Epilogue is a single ACT pass: each psum tile is preloaded with ynh by a
    DVE copy, the matmuls accumulate on top (start=False), and ACT evacuates
    with out = -2*psum + xn[i] = -2*cross + yn + xn. One store per (i,j1024).
"""

import os
from contextlib import ExitStack

import ml_dtypes
import numpy as np

import concourse.bass as bass
import concourse.tile as tile
from concourse import bacc, mybir
from concourse.bass import ds, ts
from concourse.bass_utils import run_bass_kernel_spmd

N, M, D = 8192, 8192, 512
NCORES = 8
GX, GY = 4, 2
RS = N // GX      # 2048 x-rows per core
MS = M // GY      # 4096 y-rows (output cols) per core
P = 128
KC = D // P       # 4 k-subtiles of 128
KB = KC // 2      # 2 DoubleRow k-blocks of 256
NIT = RS // P     # 16 i-tiles per core
JBLK = 1024
NJ = MS // JBLK   # 4 j-chunks
NH = JBLK // 512  # psum tiles per j-chunk

F32 = mybir.dt.float32
F32R = mybir.dt.float32r
F8 = mybir.dt.float8e4
AF = mybir.ActivationFunctionType
DR = mybir.MatmulPerfMode.DoubleRow


def _build_program():
    nc = bacc.Bacc("TRN2", target_bir_lowering=False, debug=False)

    xq_d = nc.dram_tensor("xq", [D, RS], F8, kind="ExternalInput").ap()
    yq_d = nc.dram_tensor("yq", [D, MS], F8, kind="ExternalInput").ap()
    xn_d = nc.dram_tensor("xn", [P, NIT], F32, kind="ExternalInput").ap()
    yn_d = nc.dram_tensor("yn", [1, MS], F32R, kind="ExternalInput").ap()
    out_d = nc.dram_tensor("out", [RS, MS], F32, kind="ExternalOutput").ap()

    with tile.TileContext(nc) as tc, ExitStack() as ctx:
        consts = ctx.enter_context(tc.tile_pool(name="consts", bufs=1))
        opool = ctx.enter_context(tc.tile_pool(name="opool", bufs=6))
        mm_ps = ctx.enter_context(tc.tile_pool(name="mm_ps", bufs=6, space="PSUM"))

        # fp8 GEMM operands laid out [partition, k_subtile, free] so a
        # [:, 2k:2k+2, :] slice is a DoubleRow K=256 operand pair.
        # k0/k1 of both operands go first so kb=0 matmuls can start while
        # the k2/k3 halves are still in flight; the tiny norm inputs ride
        # along in the gaps.
        xq_sb = consts.tile([P, KC, RS], F8)
        yq_sb = consts.tile([P, KC, MS], F8)
        for k in (0, 1):
            nc.sync.dma_start(xq_sb[:, k, :], xq_d[ts(k, P), :])
        for k in (0, 1):
            nc.sync.dma_start(yq_sb[:, k, :], yq_d[ts(k, P), :])

        xn_sb = consts.tile([P, NIT], F32)
        nc.sync.dma_start(xn_sb, xn_d)
        yn_sb = consts.tile([1, MS], F32R)
        nc.sync.dma_start(yn_sb, yn_d)

        for k in (2, 3):
            nc.sync.dma_start(xq_sb[:, k, :], xq_d[ts(k, P), :])
        for k in (2, 3):
            nc.sync.dma_start(yq_sb[:, k, :], yq_d[ts(k, P), :])

        # ynh[p, j] = -0.5 * yn[j], all partitions: K=1 matmul broadcast.
        ones_mh = consts.tile([1, P], F32)
        nc.vector.memset(ones_mh, -0.5)
        ones_mh_r = consts.tile([1, P], F32R)
        nc.vector.tensor_copy(ones_mh_r, ones_mh)
        ynh_sb = consts.tile([P, MS], F32)
        for q in range(MS // 512):
            ps = mm_ps.tile([P, 512], F32, tag="mm", name=f"rep{q}")
            nc.tensor.matmul(
                ps,
                ones_mh_r,
                yn_sb[0:1, ds(q * 512, 512)],
                start=True,
                stop=True,
            )
            nc.vector.tensor_copy(ynh_sb[:, ds(q * 512, 512)], ps)

        for it in range(NIT):
            for jc in range(NJ):
                pos = [
                    mm_ps.tile([P, 512], F32, tag="mm", name=f"po{it}_{jc}_{h}")
                    for h in range(NH)
                ]
                # preload psum with -yn/2; matmuls accumulate on top, so the
                # ACT evacuation's -2 scale turns it into +yn for free.
                for h in range(NH):
                    nc.vector.tensor_copy(
                        pos[h], ynh_sb[:, ds(jc * JBLK + h * 512, 512)]
                    )
                for kb in range(KB):
                    for h in range(NH):
                        nc.tensor.matmul(
                            pos[h],
                            xq_sb[:, 2 * kb : 2 * kb + 2, ts(it, P)],
                            yq_sb[:, 2 * kb : 2 * kb + 2, ds(jc * JBLK + h * 512, 512)],
                            start=False,
                            stop=(kb == KB - 1),
                            perf_mode=DR,
                            skip_group_check=True,
                        )
                o_sb = opool.tile([P, JBLK], F32, tag="o")
                for h in range(NH):
                    nc.scalar.activation(
                        o_sb[:, ds(h * 512, 512)],
                        pos[h],
                        AF.Identity,
                        bias=xn_sb[:, it : it + 1],
                        scale=-2.0,
                    )
                nc.sync.dma_start(out_d[ts(it, P), ds(jc * JBLK, JBLK)], o_sb)

    nc.compile()
    return nc


_PROGRAM = None


def _program():
    global _PROGRAM
    if _PROGRAM is None:
        _PROGRAM = _build_program()
    return _PROGRAM


def make_in_maps(x, y, log_scale):
    x = np.asarray(x, dtype=np.float32)
    y = np.asarray(y, dtype=np.float32)
    s = np.exp(np.asarray(log_scale, dtype=np.float32))

    qx = (x * s).astype(ml_dtypes.float8_e4m3)  # (N, D)
    qy = (y * s).astype(ml_dtypes.float8_e4m3)  # (M, D)
    fx = qx.astype(np.float32)
    fy = qy.astype(np.float32)
    xn = np.einsum("nd,nd->n", fx, fx)  # (N,)
    yn = np.einsum("md,md->m", fy, fy)  # (M,)

    qxT = np.ascontiguousarray(qx.T)  # (D, N)
    qyT = np.ascontiguousarray(qy.T)  # (D, M)

    in_maps = []
    for c in range(NCORES):
        a, b = c // GY, c % GY
        # xn in p-major layout: xn_c[p, it] = xn[a*RS + it*128 + p]
        xn_c = xn[a * RS : (a + 1) * RS].reshape(NIT, P).T
        in_maps.append(
            {
                "xq": np.ascontiguousarray(qxT[:, a * RS : (a + 1) * RS]),
                "yq": np.ascontiguousarray(qyT[:, b * MS : (b + 1) * MS]),
                "xn": np.ascontiguousarray(xn_c),
                "yn": np.ascontiguousarray(yn[b * MS : (b + 1) * MS]).reshape(1, MS),
            }
        )
    return in_maps


def kernel(x, y, log_scale, **_):
    nc = _program()
    in_maps = make_in_maps(x, y, log_scale)
    res = run_bass_kernel_spmd(nc, in_maps, list(range(NCORES)))
    out = np.empty((N, M), dtype=np.float32)
    for c in range(NCORES):
        a, b = c // GY, c % GY
        out[a * RS : (a + 1) * RS, b * MS : (b + 1) * MS] = res.results[c]["out"]
    return out
